# revision 9
# baseline (speedup 1.0000x reference)
"""AdaptiveAttentionLayer on 8 TRN2 NeuronCores.

Full inputs in, full output out. Sharding: data-parallel over batch (B=4)
x 2-way sequence-parallel over the 4096 query rows -> 8 cores, each core
computes a [2048, 256] slice of one batch item's output.

Per-core device pipeline (all channel-major / transposed layouts):
  - instance-norm stats of content/style (free-axis reductions)
  - V = style @ Wv              (raw style; bias broadcast-added)
  - K = style @ (diag(inv_s) Wk) + folded bias   (norm folded into weights)
  - Q = norm_content @ Wq
  - Qn, Kn = l2norm rows; PE-transpose to channel-major
  - scores^T[k,q] = Kn Qn^T  (fp32r matmuls, 512-query chunks)
  - P = exp(scores)  (cosine scores in [-1,1]: no max subtraction needed)
  - M^T = V^T P^T, E2^T = (V*V)^T P^T accumulated over key tiles
  - r = sum_k P (gpsimd partition_all_reduce), out = sqrt(relu(E2/r-(M/r)^2))
        * norm_content + M/r
"""

import sys

if "/opt/trn_rl_repo" not in sys.path:
    sys.path.insert(0, "/opt/trn_rl_repo")

import os
import numpy as np

import concourse.bass as bass
import concourse.mybir as mybir
import concourse.tile as tile
from concourse.masks import make_identity
from concourse.bass_utils import run_bass_kernel_spmd

F32 = mybir.dt.float32
F32R = mybir.dt.float32r
ALU = mybir.AluOpType
ACTF = mybir.ActivationFunctionType

B, H, W, C = 4, 64, 64, 256
N = H * W          # 4096 key/query rows per batch item
QH = N // 2        # 2048 query rows per core
NK = N // 128      # 32 key tiles
NQ = QH // 128     # 16 query tiles per core
QC = 512           # query chunk (matmul moving free dim)
NQC = QH // QC     # 4 query chunks per core
EPS_IN = 1e-5      # instance norm eps
EPS_L2 = 1e-12     # l2norm eps

LAST_EXEC_NS = {"v": None}


def _legalize_waits(nc):
    """This walrus build accepts at most ONE sync wait per instruction
    ('Too many sync wait commands'). Hoist extra waits onto same-engine
    NOPs inserted immediately before the offending instruction."""
    fn = nc.m.functions[0]
    nfix = 0
    for bb in fn.blocks:
        i = 0
        while i < len(bb.instructions):
            inst = bb.instructions[i]
            si = inst.sync_info
            if si is not None and len(si.on_wait) > 1:
                waits = list(si.on_wait)
                for j, w in enumerate(waits[:-1]):
                    nop = mybir.InstNoOp(
                        name=nc.get_next_instruction_name(), ins=[], outs=[]
                    )
                    nop.engine = inst.engine
                    nop.sync_info = mybir.SyncInfo(on_wait=[w], on_update=[])
                    nc.register_instruction(nop)
                    bb.instructions.insert(i + j, nop)
                i += len(waits) - 1
                inst.sync_info = mybir.SyncInfo(
                    on_wait=[waits[-1]], on_update=list(si.on_update)
                )
                nfix += 1
            i += 1
    return nfix


def _install_profshim():
    """antenv.axon_hooks is absent in this image; provide it (ctypes into
    libaxon_pjrt.so) plus an offline-safe upload_artifacts so trace=True
    yields exec_time_ns."""
    import contextlib, ctypes, types

    if "antenv.axon_hooks" in sys.modules:
        return
    so = "/opt/axon/libaxon_pjrt.so"
    hook = None
    if os.path.exists(so):
        lib = ctypes.CDLL(so)
        if hasattr(lib, "axon_start_nrt_profile"):
            lib.axon_start_nrt_profile.argtypes = [
                ctypes.POINTER(ctypes.c_int64),
                ctypes.c_size_t,
            ]
            lib.axon_start_nrt_profile.restype = ctypes.c_int64
            lib.axon_stop_nrt_profile.argtypes = [ctypes.c_char_p]
            lib.axon_stop_nrt_profile.restype = ctypes.c_int64

            @contextlib.contextmanager
            def _hook(output_dir, device_ids):
                import jax

                jax.devices()
                if device_ids:
                    ids = (ctypes.c_int64 * len(device_ids))(*device_ids)
                    rc = lib.axon_start_nrt_profile(ids, len(device_ids))
                else:
                    rc = lib.axon_start_nrt_profile(None, 0)
                if rc != 0:
                    raise RuntimeError(f"axon_start_nrt_profile rc={rc}")
                try:
                    yield
                finally:
                    n = lib.axon_stop_nrt_profile(str(output_dir).encode())
                    print(f"profile: {n} ntff file(s) -> {output_dir}",
                          file=sys.stderr)

            hook = _hook

    mod = types.ModuleType("antenv.axon_hooks")
    mod.get_axon_ntff_profile_hook = lambda: hook
    mod.set_axon_ntff_profile_hook = lambda h: None
    sys.modules["antenv.axon_hooks"] = mod

    import concourse.bass_utils as bu

    bu.upload_artifacts = lambda tmpdir: tmpdir


def _r(ap):
    return ap.bitcast(F32R)


def build_nc():
    nc = bass.Bass()

    xa_e = nc.declare_dram_parameter("xa", [C, QH], F32, isOutput=False)
    xb_e = nc.declare_dram_parameter("xb", [C, QH], F32, isOutput=False)
    st_e = nc.declare_dram_parameter("st", [C, N], F32, isOutput=False)
    wq_e = nc.declare_dram_parameter("wq", [C, C], F32, isOutput=False)
    wk_e = nc.declare_dram_parameter("wk", [C, C], F32, isOutput=False)
    wv_e = nc.declare_dram_parameter("wv", [C, C], F32, isOutput=False)
    bq_e = nc.declare_dram_parameter("bq", [1, C], F32, isOutput=False)
    bk_e = nc.declare_dram_parameter("bk", [1, C], F32, isOutput=False)
    bv_e = nc.declare_dram_parameter("bv", [1, C], F32, isOutput=False)
    out_e = nc.declare_dram_parameter("out", [C, QH], F32, isOutput=True)

    with tile.TileContext(nc) as tc:
        with tc.tile_pool(name="persist", bufs=1) as pp:
            # ---------- persistent tiles (~111 KB/partition) ----------
            ident = pp.tile([128, 128], F32)
            ones_row = pp.tile([1, 128], F32)
            ones_col = pp.tile([128, 1], F32)
            eps_in_t = pp.tile([128, 1], F32)
            eps_l2_t = pp.tile([128, 1], F32)
            wq_s = [pp.tile([128, C], F32, name=f"wq{i}") for i in range(2)]
            wk_s = [pp.tile([128, C], F32, name=f"wk{i}") for i in range(2)]
            wv_s = [pp.tile([128, C], F32, name=f"wv{i}") for i in range(2)]
            b_row = {k: pp.tile([1, C], F32, name=f"b_{k}") for k in "qkv"}
            bqb = pp.tile([128, C], F32)
            bkb = pp.tile([128, C], F32)
            bvb = pp.tile([128, C], F32)
            bk_eff = pp.tile([1, C], F32)
            knt = [pp.tile([128, N], F32, name=f"knt{i}") for i in range(2)]
            qnt = [pp.tile([128, QH], F32, name=f"qnt{i}") for i in range(2)]
            nct = [pp.tile([128, QH], F32, name=f"nct{i}") for i in range(2)]
            v_all = pp.tile([128, NK * C], F32)    # 32 x [128k, 256c]
            mean_s = [pp.tile([128, 1], F32, name=f"ms{i}") for i in range(2)]
            inv_s = [pp.tile([128, 1], F32, name=f"is{i}") for i in range(2)]
            mean_x = [pp.tile([128, 1], F32, name=f"mx{i}") for i in range(2)]
            inv_x = [pp.tile([128, 1], F32, name=f"ix{i}") for i in range(2)]

            make_identity(nc, ident[:])
            nc.vector.memset(ones_row[:], 1.0)
            nc.vector.memset(ones_col[:], 1.0)
            nc.vector.memset(eps_in_t[:], EPS_IN)
            nc.vector.memset(eps_l2_t[:], EPS_L2)

            # ================= phase 1: stats + projections =================
            with (
                tc.tile_pool(name="inputs", bufs=1) as tp,
                tc.tile_pool(name="w1", bufs=2) as w1,
                tc.tile_pool(name="psum1", bufs=2, space="PSUM") as ps1,
            ):
                st_t = [tp.tile([128, N], F32, name=f"st{i}") for i in range(2)]
                xa_t = [tp.tile([128, QH], F32, name=f"xa{i}") for i in range(2)]
                xb_t = [tp.tile([128, QH], F32, name=f"xb{i}") for i in range(2)]
                DCH = 1024  # DMA chunk along free dim for pipelining
                for i in range(2):
                    for j in range(0, N, DCH):
                        nc.sync.dma_start(
                            _r(st_t[i][:, j:j + DCH]),
                            _r(st_e[i * 128:(i + 1) * 128, j:j + DCH]),
                        )
                for i in range(2):
                    nc.sync.dma_start(_r(wv_s[i][:]), _r(wv_e[i * 128:(i + 1) * 128, :]))
                    nc.sync.dma_start(_r(wk_s[i][:]), _r(wk_e[i * 128:(i + 1) * 128, :]))
                    nc.sync.dma_start(_r(wq_s[i][:]), _r(wq_e[i * 128:(i + 1) * 128, :]))
                for k, e in (("q", bq_e), ("k", bk_e), ("v", bv_e)):
                    nc.sync.dma_start(b_row[k][:], e[:])
                for i in range(2):
                    for j in range(0, QH, DCH):
                        nc.sync.dma_start(
                            xa_t[i][:, j:j + DCH],
                            xa_e[i * 128:(i + 1) * 128, j:j + DCH],
                        )
                        nc.sync.dma_start(
                            xb_t[i][:, j:j + DCH],
                            xb_e[i * 128:(i + 1) * 128, j:j + DCH],
                        )

                def broadcast_row(row, dst):
                    ps = ps1.tile([128, C], F32, name="bc", tag="prj")
                    nc.tensor.matmul(ps[:], ones_row[:], row[:])
                    nc.vector.tensor_copy(dst[:], ps[:])

                broadcast_row(b_row["q"], bqb)
                broadcast_row(b_row["v"], bvb)

                # ---- instance-norm stats (free-axis sums over all columns)
                def stats(col_tiles, mean, inv):
                    # col_tiles: per channel-half list of [128, width] tiles
                    for i in range(2):
                        tiles = col_tiles[i]
                        nchunk = sum(t.shape[1] for t in tiles) // DCH
                        parts = w1.tile([128, nchunk], F32, name="parts")
                        sums = w1.tile([128, len(tiles)], F32, name="sums")
                        jj = 0
                        for ti, t in enumerate(tiles):
                            for j in range(0, t.shape[1], DCH):
                                scr = w1.tile([128, DCH], F32, name="sqscr",
                                              bufs=3)
                                nc.vector.scalar_tensor_tensor(
                                    out=scr[:],
                                    in0=t[:, j:j + DCH],
                                    scalar=1.0,
                                    in1=t[:, j:j + DCH],
                                    op0=ALU.mult,
                                    op1=ALU.mult,
                                    accum_out=parts[:, jj:jj + 1],
                                )
                                jj += 1
                            nc.vector.reduce_sum(
                                sums[:, ti:ti + 1], t[:],
                                axis=mybir.AxisListType.X,
                            )
                        ssq = w1.tile([128, 1], F32, name="ssq")
                        nc.vector.reduce_sum(ssq[:], parts[:],
                                             axis=mybir.AxisListType.X)
                        ssum = w1.tile([128, 1], F32, name="ssum")
                        nc.vector.reduce_sum(ssum[:], sums[:],
                                             axis=mybir.AxisListType.X)
                        nc.vector.tensor_scalar_mul(mean[i][:], ssum[:], 1.0 / N)
                        ex2 = w1.tile([128, 1], F32, name="ex2")
                        nc.vector.tensor_scalar_mul(ex2[:], ssq[:], 1.0 / N)
                        msq = w1.tile([128, 1], F32, name="msq")
                        nc.vector.tensor_mul(msq[:], mean[i][:], mean[i][:])
                        var = w1.tile([128, 1], F32, name="var")
                        nc.vector.tensor_sub(var[:], ex2[:], msq[:])
                        std = w1.tile([128, 1], F32, name="std")
                        nc.scalar.activation(std[:], var[:], ACTF.Sqrt,
                                             bias=eps_in_t[:])
                        nc.vector.reciprocal(inv[i][:], std[:])

                stats([[st_t[0]], [st_t[1]]], mean_s, inv_s)
                stats([[xa_t[0], xb_t[0]], [xa_t[1], xb_t[1]]], mean_x, inv_x)

                # norm_content^T for this core's query half
                for i in range(2):
                    nc.vector.tensor_scalar(
                        out=_r(nct[i][:]),
                        in0=xa_t[i][:],
                        scalar1=mean_x[i][:],
                        scalar2=inv_x[i][:],
                        op0=ALU.subtract,
                        op1=ALU.mult,
                    )

                # ---- fold instance norm of style into Wk
                for i in range(2):
                    nc.vector.tensor_scalar_mul(_r(wk_s[i][:]), wk_s[i][:],
                                                inv_s[i][:])
                mu_inv = [w1.tile([128, 1], F32, name=f"mi{i}")
                          for i in range(2)]
                for i in range(2):
                    nc.vector.tensor_mul(mu_inv[i][:], mean_s[i][:],
                                         inv_s[i][:])
                ps_bk = ps1.tile([1, C], F32, name="ps_bk", tag="prj")
                nc.tensor.matmul(ps_bk[:], mu_inv[0][:], wk_s[0][:],
                                 start=True, stop=False)
                nc.tensor.matmul(ps_bk[:], mu_inv[1][:], wk_s[1][:],
                                 start=False, stop=True)
                nc.vector.tensor_sub(bk_eff[:], b_row["k"][:], ps_bk[:])
                broadcast_row(bk_eff, bkb)

                # ---- V / K projections over 32 key tiles
                def l2norm_rows(src_psum, bias_bc, dst_t, dst_sl):
                    """src_psum [128,C] + bias -> l2-normalized rows,
                    transposed into dst_t[ci][:, dst_sl]."""
                    sb = w1.tile([128, C], F32, name="prj_sb", bufs=3)
                    nc.vector.tensor_add(sb[:], src_psum[:], bias_bc[:])
                    ss = w1.tile([128, 1], F32, name="prj_ss", bufs=3)
                    scr = w1.tile([128, C], F32, name="prj_scr", bufs=3)
                    nc.vector.scalar_tensor_tensor(
                        out=scr[:], in0=sb[:], scalar=1.0, in1=sb[:],
                        op0=ALU.mult, op1=ALU.mult, accum_out=ss[:],
                    )
                    std = w1.tile([128, 1], F32, name="prj_std", bufs=3)
                    nc.scalar.activation(std[:], ss[:], ACTF.Sqrt,
                                         bias=eps_l2_t[:])
                    pinv = w1.tile([128, 1], F32, name="prj_inv", bufs=3)
                    nc.vector.reciprocal(pinv[:], std[:])
                    pn = w1.tile([128, C], F32, name="prj_n", bufs=3)
                    nc.vector.tensor_scalar_mul(pn[:], sb[:], pinv[:])
                    for ci in range(2):
                        ps_t = ps1.tile([128, 128], F32, name="ps_t",
                                        tag="ps_t")
                        nc.tensor.transpose(
                            ps_t[:], pn[:, ci * 128:(ci + 1) * 128], ident[:]
                        )
                        nc.vector.tensor_copy(_r(dst_t[ci][:, dst_sl]), ps_t[:])

                for kt in range(NK):
                    ksl = slice(kt * 128, (kt + 1) * 128)
                    ps_v = ps1.tile([128, C], F32, name="ps_v", tag="prj")
                    nc.tensor.matmul(ps_v[:], _r(st_t[0][:, ksl]),
                                     _r(wv_s[0][:]), start=True, stop=False)
                    nc.tensor.matmul(ps_v[:], _r(st_t[1][:, ksl]),
                                     _r(wv_s[1][:]), start=False, stop=True)
                    vsl = slice(kt * C, (kt + 1) * C)
                    nc.vector.tensor_add(_r(v_all[:, vsl]), ps_v[:], bvb[:])

                    ps_k = ps1.tile([128, C], F32, name="ps_k", tag="prj")
                    nc.tensor.matmul(ps_k[:], _r(st_t[0][:, ksl]),
                                     _r(wk_s[0][:]), start=True, stop=False)
                    nc.tensor.matmul(ps_k[:], _r(st_t[1][:, ksl]),
                                     _r(wk_s[1][:]), start=False, stop=True)
                    l2norm_rows(ps_k, bkb, knt, ksl)

                # ---- Q projection over 16 query tiles
                for qt in range(NQ):
                    qsl = slice(qt * 128, (qt + 1) * 128)
                    ps_q = ps1.tile([128, C], F32, name="ps_q", tag="prj")
                    nc.tensor.matmul(ps_q[:], _r(nct[0][:, qsl]),
                                     _r(wq_s[0][:]), start=True, stop=False)
                    nc.tensor.matmul(ps_q[:], _r(nct[1][:, qsl]),
                                     _r(wq_s[1][:]), start=False, stop=True)
                    l2norm_rows(ps_q, bqb, qnt, qsl)

            # ================= phase 2: attention =================
            with (
                tc.tile_pool(name="w2", bufs=2) as w2,
                tc.tile_pool(name="psum_acc", bufs=1, space="PSUM") as psa,
                tc.tile_pool(name="psum_sc", bufs=2, space="PSUM") as pss,
            ):
                for qc in range(NQC):
                    qsl = slice(qc * QC, (qc + 1) * QC)
                    ps_m = [psa.tile([128, QC], F32, name=f"ps_m{c}")
                            for c in range(2)]
                    ps_e = [psa.tile([128, QC], F32, name=f"ps_e{c}")
                            for c in range(2)]
                    racc = w2.tile([128, QC], F32, name="racc")
                    for kt in range(NK):
                        ksl = slice(kt * 128, (kt + 1) * 128)
                        ps_s = pss.tile([128, QC], F32, name="ps_s")
                        nc.tensor.matmul(ps_s[:], _r(knt[0][:, ksl]),
                                         _r(qnt[0][:, qsl]),
                                         start=True, stop=False)
                        nc.tensor.matmul(ps_s[:], _r(knt[1][:, ksl]),
                                         _r(qnt[1][:, qsl]),
                                         start=False, stop=True)
                        p_sb = w2.tile([128, QC], F32, name="p_sb", bufs=3)
                        nc.scalar.activation(_r(p_sb[:]), ps_s[:], ACTF.Exp)
                        if kt == 0:
                            nc.vector.tensor_copy(_r(racc[:]), p_sb[:])
                        else:
                            nc.vector.tensor_add(_r(racc[:]), racc[:], p_sb[:])
                        v2t = w2.tile([128, C], F32, name="v2t", bufs=4)
                        vsl = slice(kt * C, (kt + 1) * C)
                        nc.vector.tensor_mul(_r(v2t[:]), v_all[:, vsl],
                                             v_all[:, vsl])
                        first, last = kt == 0, kt == NK - 1
                        for ci in range(2):
                            cs = slice(kt * C + ci * 128,
                                       kt * C + (ci + 1) * 128)
                            nc.tensor.matmul(ps_m[ci][:], _r(v_all[:, cs]),
                                             _r(p_sb[:]),
                                             start=first, stop=last)
                            nc.tensor.matmul(
                                ps_e[ci][:],
                                _r(v2t[:, ci * 128:(ci + 1) * 128]),
                                _r(p_sb[:]), start=first, stop=last)
                    # softmax denominator: PE ones-matmul partition sum,
                    # reciprocal on one row, PE broadcast back to 128 rows
                    ps_r = psa.tile([1, QC], F32, name="ps_r")
                    nc.tensor.matmul(ps_r[:], ones_col[:], racc[:])
                    r_row = w2.tile([1, QC], F32, name="r_row")
                    nc.vector.tensor_copy(r_row[:], ps_r[:])
                    rinv_row = w2.tile([1, QC], F32, name="rinv_row")
                    nc.vector.reciprocal(rinv_row[:], r_row[:])
                    ps_rb = psa.tile([128, QC], F32, name="ps_rb")
                    nc.tensor.matmul(ps_rb[:], ones_row[:], rinv_row[:])
                    rinv = w2.tile([128, QC], F32, name="rinv")
                    nc.vector.tensor_copy(rinv[:], ps_rb[:])
                    for ci in range(2):
                        mhat = w2.tile([128, QC], F32, name="mhat")
                        nc.vector.tensor_mul(mhat[:], ps_m[ci][:], rinv[:])
                        ehat = w2.tile([128, QC], F32, name="ehat")
                        nc.vector.tensor_mul(ehat[:], ps_e[ci][:], rinv[:])
                        s2 = w2.tile([128, QC], F32, name="s2")
                        nc.vector.tensor_mul(s2[:], mhat[:], mhat[:])
                        nc.vector.tensor_sub(s2[:], ehat[:], s2[:])
                        nc.vector.tensor_scalar_max(s2[:], s2[:], 0.0)
                        s_sb = w2.tile([128, QC], F32, name="s_sb")
                        nc.scalar.activation(s_sb[:], s2[:], ACTF.Sqrt)
                        o_sb = w2.tile([128, QC], F32, name="o_sb")
                        nc.vector.tensor_mul(o_sb[:], s_sb[:], nct[ci][:, qsl])
                        nc.vector.tensor_add(o_sb[:], o_sb[:], mhat[:])
                        nc.sync.dma_start(
                            out_e[ci * 128:(ci + 1) * 128, qsl], o_sb[:]
                        )

    _legalize_waits(nc)
    return nc


_NC_CACHE = {}


def _get_nc():
    if "nc" not in _NC_CACHE:
        _NC_CACHE["nc"] = build_nc()
    return _NC_CACHE["nc"]


def kernel(content, style, Wq, bq, Wk, bk, Wv, bv):
    content = np.asarray(content, dtype=np.float32)
    style = np.asarray(style, dtype=np.float32)
    Wq = np.ascontiguousarray(np.asarray(Wq, dtype=np.float32))
    Wk = np.ascontiguousarray(np.asarray(Wk, dtype=np.float32))
    Wv = np.ascontiguousarray(np.asarray(Wv, dtype=np.float32))
    bq = np.asarray(bq, dtype=np.float32).reshape(1, C)
    bk = np.asarray(bk, dtype=np.float32).reshape(1, C)
    bv = np.asarray(bv, dtype=np.float32).reshape(1, C)

    nc = _get_nc()
    in_maps = []
    for core in range(8):
        b, h = core // 2, core % 2
        xt = np.ascontiguousarray(content[b].reshape(N, C).T)
        st = np.ascontiguousarray(style[b].reshape(N, C).T)
        xa = np.ascontiguousarray(xt[:, h * QH:(h + 1) * QH])
        xb = np.ascontiguousarray(xt[:, (1 - h) * QH:(2 - h) * QH])
        in_maps.append({
            "xa": xa, "xb": xb, "st": st,
            "wq": Wq, "wk": Wk, "wv": Wv,
            "bq": bq, "bk": bk, "bv": bv,
        })

    trace = os.environ.get("BASS_KERNEL_TRACE", "0") == "1"
    if trace:
        _install_profshim()
    res = run_bass_kernel_spmd(nc, in_maps, list(range(8)), trace=trace)
    LAST_EXEC_NS["v"] = res.exec_time_ns

    out = np.empty((B, H, W, C), dtype=np.float32)
    for core in range(8):
        b, h = core // 2, core % 2
        o = res.results[core]["out"]          # [C, QH]
        out[b].reshape(N, C)[h * QH:(h + 1) * QH, :] = o.T
    return out
